# revision 24
# baseline (speedup 1.0000x reference)
"""Trainium2 Bass kernel for DampedAttention.

Full inputs in, full output out. Sharding: 8 cores = 2 batches x 4 head-groups
(4 heads of dim 64 each per core). Per core:

  QT/KT  [c, s] transposed projections (c on partitions), scale 1/8 and biases
         folded in (bias via K=1 ones-row matmuls, scale into weights on host)
  V      [s, c] natural projection (lhsT for the P@V matmul)
  ST     scores transposed [k, q] per (k-chunk, q-block) so exp(ST) is directly
         the lhsT-layout P^T needed by P@V -- no on-chip transposes
  ctxT   [65, q] = V_aug^T @ P^T ; row 64 = softmax row-sums (ones column in V)
  LVT    [64, q] banded 0.4*L^T matmuls (8 unique host-built band tiles)
  blend  ctxT_final = PV * (0.6/r, bcast over partitions) + LVT
  out    [s, o] natural out-projection; host sums 4 head-group partials + bo

Matmul operands are bf16 (fp32 matmul lowers to a 2x HI/LO instruction pair on
TRN2); accumulation, softmax row-sums, reciprocal and the 0.6/r normalization
stay fp32. The entropy gate in the reference is a forward no-op and is
skipped. Softmax max-subtraction is skipped (scores are O(1), no overflow).
"""
import numpy as np
import ml_dtypes

S = 2048
D = 1024
CLOC = 256          # channels per core (4 heads x 64)
HD = 64
NH = 4              # heads per core
NDC = 8             # 128-wide d-chunks in contraction D
NKC = 16            # 128-wide k/s chunks in S
NQB = 4             # 512-wide q blocks
QB = 512
WINDOW = 3
STRENGTH = 0.4
EPS = 1e-10
F32 = np.float32
BF16 = ml_dtypes.bfloat16


def _build_L04T():
    i = np.arange(S)
    d = (i[:, None] - i[None, :]).astype(F32)
    k = np.where(np.abs(d) <= WINDOW,
                 np.exp(-(d ** 2) / F32(2.0 * STRENGTH ** 2)),
                 F32(0.0)).astype(F32)
    L = k / (k.sum(axis=-1, keepdims=True) + F32(EPS))
    return (F32(0.4) * L).T.copy()  # [s, q], pre-scaled by (1 - lambda_jump)


def _lt_tiles():
    """Unique [128, 512] band tiles of 0.4*L^T plus (qb -> [(j, uniq_idx)])."""
    L04T = _build_L04T()
    uniq = []
    slots = {qb: [] for qb in range(NQB)}
    for qb in range(NQB):
        for j in range(max(0, qb * 4 - 1), min(NKC, qb * 4 + 5)):
            t = L04T[j * 128:(j + 1) * 128, qb * QB:(qb + 1) * QB]
            for ui, ut in enumerate(uniq):
                if np.array_equal(t, ut):
                    slots[qb].append((j, ui))
                    break
            else:
                slots[qb].append((j, len(uniq)))
                uniq.append(t)
    return np.stack(uniq).astype(BF16), slots


_LT_UNIQ, _LT_SLOTS = _lt_tiles()
NU = _LT_UNIQ.shape[0]

_CACHE = {}


def _build_program():
    import concourse.bacc as bacc
    import concourse.mybir as mybir
    from concourse.tile import TileContext
    from concourse.bass_isa import ReduceOp  # noqa: F401  (engine availability)

    f32 = mybir.dt.float32
    bf16 = mybir.dt.bfloat16
    Exp = mybir.ActivationFunctionType.Exp

    nc = bacc.Bacc("TRN2", target_bir_lowering=False, debug=False,
                   enable_asserts=False, num_devices=8)

    xt = nc.dram_tensor("xt", [D, S], bf16, kind="ExternalInput").ap()
    wqt = nc.dram_tensor("wqt", [D, CLOC], bf16, kind="ExternalInput").ap()
    wkt = nc.dram_tensor("wkt", [D, CLOC], bf16, kind="ExternalInput").ap()
    wvt = nc.dram_tensor("wvt", [D, CLOC], bf16, kind="ExternalInput").ap()
    bqr = nc.dram_tensor("bqr", [1, CLOC], bf16, kind="ExternalInput").ap()
    bkr = nc.dram_tensor("bkr", [1, CLOC], bf16, kind="ExternalInput").ap()
    bvr = nc.dram_tensor("bvr", [1, CLOC], bf16, kind="ExternalInput").ap()
    wot = nc.dram_tensor("wot", [CLOC, D], bf16, kind="ExternalInput").ap()
    ltt = nc.dram_tensor("ltt", [NU, 128, QB], bf16, kind="ExternalInput").ap()
    out = nc.dram_tensor("out", [S, D], f32, kind="ExternalOutput").ap()

    with TileContext(nc) as tc:
        with tc.tile_pool(name="persist", bufs=1) as pp:
            # ---- persistent SBUF ----
            # per-c-tile tensors so head-pair 0 attention is not
            # dependency-gated on c-tile 1 projections
            qt = [pp.tile([128, S], bf16, name=f"qt{i}") for i in range(2)]
            kt = [pp.tile([128, S], bf16, name=f"kt{i}") for i in range(2)]
            v_all = pp.tile([128, NKC, NH, HD + 1], bf16)  # ones col at 64
            ctxt_all = pp.tile([128, 2, S], bf16)
            wot_sb = pp.tile([128, 2, D], bf16)
            bq_sb = pp.tile([1, CLOC], bf16)
            bk_sb = pp.tile([1, CLOC], bf16)
            bv_sb = pp.tile([1, CLOC], bf16)
            ones_r = pp.tile([1, QB], bf16)          # ones row (bias outer prod)
            ones_c = pp.tile([1, 128], bf16)         # ones row (V bias)

            nc.gpsimd.memset(ones_r[:], 1.0)
            nc.gpsimd.memset(ones_c[:], 1.0)
            nc.gpsimd.memset(v_all[:, :, :, HD:HD + 1], 1.0)

            nc.sync.dma_start(bq_sb[:], bqr[:])
            nc.sync.dma_start(bk_sb[:], bkr[:])
            nc.sync.dma_start(bv_sb[:], bvr[:])
            for cc in range(2):
                nc.sync.dma_start(wot_sb[:, cc, :], wot[cc * 128:(cc + 1) * 128, :])

            # ---- phase B: projections ----
            with (
                tc.tile_pool(name="projsb", bufs=1) as prs,
                tc.tile_pool(name="projps", bufs=4, space="PSUM") as prp,
                tc.tile_pool(name="vps", bufs=2, space="PSUM") as vpp,
            ):
                xt_sb = prs.tile([128, NDC, S], bf16)
                for dc in range(NDC):
                    nc.sync.dma_start(xt_sb[:, dc, :],
                                      xt[dc * 128:(dc + 1) * 128, :])
                wq_sb = prs.tile([128, NDC, CLOC], bf16)
                wk_sb = prs.tile([128, NDC, CLOC], bf16)
                wv_sb = prs.tile([128, NDC, CLOC], bf16)
                for dc in range(NDC):
                    nc.sync.dma_start(wq_sb[:, dc, :], wqt[dc * 128:(dc + 1) * 128, :])
                    nc.sync.dma_start(wk_sb[:, dc, :], wkt[dc * 128:(dc + 1) * 128, :])
                    nc.sync.dma_start(wv_sb[:, dc, :], wvt[dc * 128:(dc + 1) * 128, :])

                # V natural first (attention needs all of V before any P@V):
                # [s-chunk 128, 256], contraction over d
                for sc in range(NKC):
                    ps = vpp.tile([128, CLOC], f32, tag="vps")
                    for dc in range(NDC):
                        nc.tensor.matmul(
                            ps[:],
                            xt_sb[:, dc, sc * 128:(sc + 1) * 128],
                            wv_sb[:, dc, :],
                            start=(dc == 0), stop=False)
                    nc.tensor.matmul(ps[:], ones_c[:], bv_sb[:],
                                     start=False, stop=True)
                    nc.vector.tensor_copy(
                        v_all[:, sc, :, 0:HD],
                        ps[:].rearrange("p (h e) -> p h e", h=NH))

                # QT / KT: [c-tile 128, s-block 512], contraction over d.
                # ct outermost so head-pair 0 attention can start after ct 0;
                # qb innermost so one weight load serves 4 matmuls.
                for ct in range(2):
                    for dst, w_sb, b_sb in ((qt[ct], wq_sb, bq_sb),
                                            (kt[ct], wk_sb, bk_sb)):
                        pss = [prp.tile([128, QB], f32, tag="projps", name=f"pjps{qb}")
                               for qb in range(NQB)]
                        for dc in range(NDC):
                            for qb in range(NQB):
                                nc.tensor.matmul(
                                    pss[qb][:],
                                    w_sb[:, dc, ct * 128:(ct + 1) * 128],
                                    xt_sb[:, dc, qb * QB:(qb + 1) * QB],
                                    start=(dc == 0), stop=False)
                        for qb in range(NQB):
                            nc.tensor.matmul(
                                pss[qb][:], b_sb[:, ct * 128:(ct + 1) * 128],
                                ones_r[:], start=False, stop=True)
                            nc.vector.tensor_copy(
                                dst[:, qb * QB:(qb + 1) * QB], pss[qb][:])

            # ---- phase C: attention per (head-pair, q-block) ----
            # Heads 2hp/2hp+1 live at partitions 0-63/64-127 of c-tile hp, so
            # interleaving their score matmuls alternates PE row-groups
            # (weight loads overlap compute) and keeps PE dense for HAM.
            with (
                tc.tile_pool(name="attnsb", bufs=1) as ab,
                tc.tile_pool(name="stage", bufs=3) as sp,
                tc.tile_pool(name="pt", bufs=10) as ptp,
                tc.tile_pool(name="stps", bufs=3, space="PSUM") as stp,
                tc.tile_pool(name="ctxps", bufs=2, space="PSUM") as ctp,
            ):
                lt_sb = ab.tile([128, NU, QB], bf16)
                for u in range(NU):
                    nc.sync.dma_start(lt_sb[:, u, :], ltt[u, :, :])
                mult = mybir.AluOpType.mult
                add = mybir.AluOpType.add
                for hp in range(2):
                    for qb in range(NQB):
                        qsl = slice(qb * QB, (qb + 1) * QB)
                        ctx = [ctp.tile([128, QB], f32, tag="ctxps", name=f"ctx{hh}")
                               for hh in range(2)]
                        for kc in range(NKC):
                            st_ps = stp.tile([128, 2, QB], f32, tag="stps")
                            for hh in range(2):
                                p0 = hh * 64
                                # explicit tile_position: K=64 row-group
                                # packing so the head pair runs concurrently
                                nc.tensor.matmul(
                                    st_ps[:, hh, :],
                                    kt[hp][p0:p0 + 64, kc * 128:(kc + 1) * 128],
                                    qt[hp][p0:p0 + 64, qsl],
                                    start=True, stop=True,
                                    tile_position=(p0, 0))
                            pt_sb = ptp.tile([128, 2, QB], bf16, tag="pt")
                            nc.scalar.activation(pt_sb[:], st_ps[:], Exp)
                            for hh in range(2):
                                nc.tensor.matmul(
                                    ctx[hh][0:HD + 1, :],
                                    v_all[:, kc, 2 * hp + hh, 0:HD + 1],
                                    pt_sb[:, hh, :],
                                    start=(kc == 0), stop=(kc == NKC - 1))
                        slots = _LT_SLOTS[qb]
                        # banded 0.4*L^T term, both heads column-packed into
                        # one psum tile (col strips 0-1 / 2-3 run concurrently)
                        lv_ps = stp.tile([128, QB], f32, tag="stps")
                        for n, (j, u) in enumerate(slots):
                            for hh in range(2):
                                nc.tensor.matmul(
                                    lv_ps[hh * HD:(hh + 1) * HD, :],
                                    v_all[:, j, 2 * hp + hh, 0:HD],
                                    lt_sb[:, u, :],
                                    start=(n == 0), stop=(n == len(slots) - 1),
                                    tile_position=(0, hh * HD),
                                    skip_group_check=True)
                        for hh in range(2):
                            h = 2 * hp + hh
                            # 1/rowsum = exp(-ln r) on ScalarE (~2x faster than
                            # the DVE reciprocal; Ln and Exp share a table set)
                            lnr = sp.tile([65, QB], f32, tag="lnr")
                            nc.scalar.activation(
                                lnr[64:65, :], ctx[hh][64:65, :],
                                mybir.ActivationFunctionType.Ln)
                            bc_src = sp.tile([1, QB], f32, tag="bcsrc")
                            nc.scalar.activation(
                                bc_src[0:1, :], lnr[64:65, :],
                                mybir.ActivationFunctionType.Exp, scale=-1.0)
                            bc_sb = sp.tile([64, QB], f32, tag="bcsb")
                            nc.gpsimd.partition_broadcast(
                                bc_sb[:], bc_src[:], channels=HD)
                            # blend: (PV/r)*0.6 + 0.4LV, staged out per q-block
                            m1 = sp.tile([64, QB], f32, tag="m1")
                            nc.vector.tensor_mul(m1[:], ctx[hh][0:HD, :], bc_sb[:])
                            stg = sp.tile([64, QB], bf16, tag="stg")
                            nc.vector.scalar_tensor_tensor(
                                stg[:], m1[:], 0.6,
                                lv_ps[hh * HD:(hh + 1) * HD, :],
                                op0=mult, op1=add)
                            nc.sync.dma_start(
                                ctxt_all[hh * 64:hh * 64 + 64, hp, qsl], stg[:])

            # ---- phase D: out-projection ----
            with (
                tc.tile_pool(name="ops", bufs=2, space="PSUM") as opp,
                tc.tile_pool(name="osb", bufs=6) as osb,
            ):
                for sc in range(NKC):
                    for ot in range(2):
                        ps = opp.tile([128, QB], f32, tag="ops")
                        for cc in range(2):
                            nc.tensor.matmul(
                                ps[:],
                                ctxt_all[:, cc, sc * 128:(sc + 1) * 128],
                                wot_sb[:, cc, ot * QB:(ot + 1) * QB],
                                start=(cc == 0), stop=(cc == 1))
                        ot_sb = osb.tile([128, QB], f32, tag="osb")
                        nc.vector.tensor_copy(ot_sb[:], ps[:])
                        nc.sync.dma_start(
                            out[sc * 128:(sc + 1) * 128, ot * QB:(ot + 1) * QB],
                            ot_sb[:])

    nc.compile()
    return nc


def _get_program():
    if "nc" not in _CACHE:
        _CACHE["nc"] = _build_program()
    return _CACHE["nc"]


def _in_maps(x, Wq, bq, Wk, bk, Wv, bv, Wo):
    xT = [np.ascontiguousarray(x[b].T).astype(BF16) for b in range(2)]
    maps = []
    for c in range(8):
        b, hg = c // 4, c % 4
        hs, he = hg * CLOC, (hg + 1) * CLOC
        maps.append({
            "xt": xT[b],
            "wqt": np.ascontiguousarray(Wq[hs:he].T / F32(8.0)).astype(BF16),
            "wkt": np.ascontiguousarray(Wk[hs:he].T).astype(BF16),
            "wvt": np.ascontiguousarray(Wv[hs:he].T).astype(BF16),
            "bqr": (bq[hs:he] / F32(8.0))[None, :].astype(BF16),
            "bkr": bk[hs:he][None, :].astype(BF16),
            "bvr": bv[hs:he][None, :].astype(BF16),
            "wot": np.ascontiguousarray(Wo[:, hs:he].T).astype(BF16),
            "ltt": _LT_UNIQ,
        })
    return maps


def _run(x, Wq, bq, Wk, bk, Wv, bv, Wo, bo, trace=False):
    from concourse.bass_utils import run_bass_kernel_spmd
    nc = _get_program()
    maps = _in_maps(np.asarray(x, F32), np.asarray(Wq, F32), np.asarray(bq, F32),
                    np.asarray(Wk, F32), np.asarray(bk, F32), np.asarray(Wv, F32),
                    np.asarray(bv, F32), np.asarray(Wo, F32))
    res = run_bass_kernel_spmd(nc, maps, list(range(8)), trace=trace)
    bo = np.asarray(bo, F32)
    outp = np.empty((2, S, D), F32)
    for b in range(2):
        acc = res.results[b * 4]["out"].astype(F32)
        for hg in range(1, 4):
            acc = acc + res.results[b * 4 + hg]["out"]
        outp[b] = acc + bo
    return outp, res


def kernel(x, Wq, bq, Wk, bk, Wv, bv, Wo, bo):
    outp, _ = _run(x, Wq, bq, Wk, bk, Wv, bv, Wo, bo, trace=False)
    return outp


def kernel_traced(**inputs):
    return _run(trace=True, **inputs)
